# revision 25
# baseline (speedup 1.0000x reference)
"""Trainium2 Bass kernel for the char-CNN NLP model (data-parallel over 8 cores).

Pipeline:
  host:   emb = x @ emb_w (one-hot projection), laid out [cin, batch, seq],
          quantized to fp8e4 (scaled x64; TRN FP8_EXP4 == ml_dtypes.float8_e4m3)
  device: 3 parallel 1-D conv banks (k=2,3,4; 256 filters each) as fp8
          DoubleRow matmuls (two cin-chunks contracted per pass, fp32 PSUM);
          per (channel, batch) max over sequence; per channel sum of squares
          -> tiny stats tensor per core
  host:   batchnorm statistics from the factorized mean + device sumsq,
          monotone-affine BN+ReLU+maxpool reconstruction from max (min when
          some bn gamma < 0), fc1 -> bn -> relu -> fc2 -> softmax

BN(c+bias) is affine per channel, so max_t relu(bn(c)) = relu(s*M + t) with
M = max_t c if s>=0 else min_t c - exact, and the conv bias cancels inside BN.

Layout trick: each batch's sequence is stored at stride 128 (= S) with no
per-batch gap, so a conv tap at offset kk is one flat contiguous 512-wide
moving operand covering 4 batches; output columns t in [L, 128) accumulate
garbage that the evacuation slices away.
"""

import os
import numpy as np
import ml_dtypes

# ---------------- problem constants (hardcoded per contract) ----------------
B, S, W, V, E = 128, 128, 16, 128, 32
FILTERS = [256, 256, 256]
KS = [2, 3, 4]
NCLS = 10
EPS = 1e-5
NCORES = 8
BL = B // NCORES             # 16 batches per core
CIN = W * E                  # 512 conv input channels
NCC = CIN // 128             # 4 contraction chunks
NPAIR = NCC // 2             # 2 DoubleRow chunk pairs
LS = [S - k + 1 for k in KS]  # 127, 126, 125 valid conv positions
XH = 8 * 128                 # one batch-half (8 batches x 128) elems
XHP = XH + 32                # padded half stride (tap reads may run 3 past)
EMB_FREE = 2 * 2 * XHP       # (h, c, x) layout per pair tile = 4224
SC_A = 64.0                  # activation fp8 scale
SC_W = 64.0                  # weight fp8 scale
# group order: last group is a k=2 bank so the final accumulation chain and
# evacuation are as short as possible
GROUPS = [(0, 0), (1, 0), (1, 1), (2, 0), (2, 1), (0, 1)]
# per-group evacuation pieces: (stat block col, nb batches); last group ends
# with 2+1+1 batches so the tail only trails by a single-batch piece
PIECES_FULL = [(0, 4), (5, 4), (10, 4), (15, 4)]
PIECES_LAST = [(0, 4), (5, 4), (10, 4), (15, 2), (18, 1), (20, 1)]
NSTAT = 27                   # 4x(4 max + 1 sq) blocks; last group's final
                             # batch stores max + bn_stats(6) instead of a sq
F8 = ml_dtypes.float8_e4m3   # TRN FP8_EXP4: bias 7, max +-240

_CACHE = {}
_LAST_RESULTS = None


def _group_tiles(bank):
    return [(ccp, kk) for ccp in range(NPAIR) for kk in range(KS[bank])]


def _weight_tile_count():
    return sum(len(_group_tiles(bank)) for bank, _ in GROUPS)


def _build_bass(need_min):
    import concourse.tile as tile
    from concourse import bacc, mybir
    from contextlib import ExitStack

    nc = bacc.Bacc("TRN2", target_bir_lowering=False, debug=False, enable_asserts=False)

    ntiles = _weight_tile_count()  # 36 DoubleRow tiles of [128, 2, 128]
    nstat = NSTAT + (16 if need_min else 0)
    DR = mybir.MatmulPerfMode.DoubleRow
    emb_d = nc.dram_tensor(
        "emb", [NPAIR, 128, EMB_FREE], mybir.dt.float8e4, kind="ExternalInput"
    ).ap()
    wts_d = nc.dram_tensor(
        "wts", [128, ntiles * 256], mybir.dt.float8e4, kind="ExternalInput"
    ).ap()
    stats_d = nc.dram_tensor(
        "stats", [len(GROUPS), 128, nstat], mybir.dt.float32, kind="ExternalOutput"
    ).ap()

    with tile.TileContext(nc) as tc, ExitStack() as ctx:
        const_pool = ctx.enter_context(tc.tile_pool(name="const", bufs=1))
        psum_pool = ctx.enter_context(tc.tile_pool(name="psum", bufs=8, space="PSUM"))
        stats_pool = ctx.enter_context(tc.tile_pool(name="stats", bufs=3))
        scr_pool = ctx.enter_context(tc.tile_pool(name="scr", bufs=4))

        # ---- PE warmup: junk DoubleRow matmuls on a zeroed tile while input
        # DMAs stream, so HAM un-throttles before the real stream starts.
        # memset on gpsimd: that engine finishes its init earliest. ----
        warm = const_pool.tile([128, 1024], mybir.dt.float8e4, name="warm")
        nc.gpsimd.memset(warm[:], 0.0)
        wlhs = warm[:, :256].rearrange("p (c f) -> p c f", c=2)
        wrhs = warm[:].rearrange("p (c x) -> p c x", c=2)
        wpsum = psum_pool.tile([128, 512], mybir.dt.float32, tag="ps", name="wps")
        for _ in range(3):
            nc.tensor.matmul(
                wpsum[:], wlhs, wrhs, start=True, stop=True, perf_mode=DR
            )

        # ---- load inputs over BOTH HWDGE queues (sync + scalar) so the
        # dispatch serialization halves and the first pieces land sooner ----
        bases = []
        base = 0
        for bank, fc in GROUPS:
            bases.append(base)
            base += len(_group_tiles(bank))
        wt_sb = [
            const_pool.tile(
                [128, len(_group_tiles(GROUPS[g][0])) * 256],
                mybir.dt.float8e4, tag=f"w{g}", name=f"w{g}",
            )
            for g in range(len(GROUPS))
        ]
        emb_sb = [
            const_pool.tile(
                [128, EMB_FREE], mybir.dt.float8e4, tag=f"e{p}", name=f"e{p}"
            )
            for p in range(NPAIR)
        ]

        def load_wt(eng, g, t0, t1):
            eng.dma_start(
                wt_sb[g][:, t0 * 256 : t1 * 256],
                wts_d[:, (bases[g] + t0) * 256 : (bases[g] + t1) * 256],
            )

        def load_emb_half(eng, p, h):
            eng.dma_start(
                emb_sb[p][:, h * 2 * XHP : (h + 1) * 2 * XHP],
                emb_d[p][:, h * 2 * XHP : (h + 1) * 2 * XHP],
            )

        # consumption order for quad-major group 0: tile0 needs wt_g0a+p0h0,
        # tiles 2-3 need wt_g0b+p1h0, quads 2-3 need the h1 halves
        nt0 = len(_group_tiles(GROUPS[0][0]))
        load_wt(nc.sync, 0, 0, 1)
        load_emb_half(nc.scalar, 0, 0)
        load_wt(nc.sync, 0, 1, nt0)
        load_emb_half(nc.scalar, 1, 0)
        load_emb_half(nc.sync, 0, 1)
        load_emb_half(nc.scalar, 1, 1)
        for g, eng in zip(range(1, len(GROUPS)),
                          [nc.sync, nc.scalar, nc.sync, nc.scalar, nc.sync]):
            load_wt(eng, g, 0, len(_group_tiles(GROUPS[g][0])))

        def rhs_ap(ccp, q, kk, boff, nb):
            # moving operand [128, 2, nb*128]: dim1 steps between the two
            # chunks of the pair, free covers nb batches at stride 128
            src = emb_sb[ccp][:].rearrange("p (h c x) -> p h c x", c=2, x=XHP)
            x0 = (q % 2) * 512 + boff * 128 + kk
            return src[:, q // 2, :, x0 : x0 + nb * 128]

        # ---- conv banks: quad-major so each accumulation group stops early
        # and its evacuation overlaps the next quad's matmul stream. The last
        # group's final 2+1+1 batches run at the very end so only a
        # single-batch evacuation + a tiny stats DMA trail the stream. ----
        st_sb = [
            stats_pool.tile([128, nstat], mybir.dt.float32, tag="st", name=f"st{g}")
            for g in range(len(GROUPS))
        ]

        def emit_piece(g, bank, col, nb, bidx, pi):
            L = LS[bank]
            tiles = _group_tiles(bank)
            wt = wt_sb[g]
            st = st_sb[g]
            q = bidx // 4
            boff = bidx - q * 4
            ps = psum_pool.tile(
                [128, nb * 128], mybir.dt.float32, tag="ps", name=f"ps{g}_{pi}"
            )
            for i, (ccp, kk) in enumerate(tiles):
                lhs = wt[:, i * 256 : (i + 1) * 256].rearrange(
                    "p (c f) -> p c f", c=2
                )
                nc.tensor.matmul(
                    ps[:], lhs, rhs_ap(ccp, q, kk, boff, nb),
                    start=(i == 0), stop=(i == len(tiles) - 1), perf_mode=DR,
                )
            pv = ps[:].rearrange("p (b t) -> p b t", t=128)[:, :, :L]
            nc.vector.tensor_reduce(
                st[:, col : col + nb], pv, axis=mybir.AxisListType.X,
                op=mybir.AluOpType.max,
            )
            if need_min:
                nc.vector.tensor_reduce(
                    st[:, NSTAT + bidx : NSTAT + bidx + nb], pv,
                    axis=mybir.AxisListType.X, op=mybir.AluOpType.min,
                )
            scr = scr_pool.tile([128, 512], mybir.dt.float32, tag="scr")
            scr_v = scr[:, : nb * L].rearrange("p (b t) -> p b t", t=L)
            nc.scalar.activation(
                scr_v, pv,
                mybir.ActivationFunctionType.Square,
                accum_out=st[:, col + nb : col + nb + 1],
            )

        def emit_group_pieces(g, pieces, bidx0):
            bank, _ = GROUPS[g]
            bidx = bidx0
            for pi, (col, nb) in enumerate(pieces):
                emit_piece(g, bank, col, nb, bidx, f"{pi}_{bidx}")
                bidx += nb

        glast = len(GROUPS) - 1
        for g in range(glast - 1):
            emit_group_pieces(g, PIECES_FULL, 0)
            nc.sync.dma_start(stats_d[g][:, 0:nstat], st_sb[g][:, 0:nstat])
        # last group's first 14 batches run before the second-to-last group,
        # so their evacuations overlap that group's stream
        emit_group_pieces(glast, PIECES_LAST[:-1], 0)
        nc.sync.dma_start(stats_d[glast][:, 0:20], st_sb[glast][:, 0:20])
        if need_min:
            nc.sync.dma_start(
                stats_d[glast][:, NSTAT : NSTAT + 15],
                st_sb[glast][:, NSTAT : NSTAT + 15],
            )
        emit_group_pieces(glast - 1, PIECES_FULL, 0)
        # group 4's stats leave on the scalar HWDGE queue: its dispatch then
        # runs concurrently with the sync queue's two trailing dispatches
        # instead of serializing ahead of them
        nc.scalar.dma_start(
            stats_d[glast - 1][:, 0:nstat], st_sb[glast - 1][:, 0:nstat]
        )
        # the final single-batch piece evacuates entirely on the vector
        # engine (max + bn_stats) so its square never queues behind group 4's
        # squares on the scalar engine
        bank5 = GROUPS[glast][0]
        L5 = LS[bank5]
        tiles5 = _group_tiles(bank5)
        psf = psum_pool.tile([128, 128], mybir.dt.float32, tag="ps", name="psfin")
        for i, (ccp, kk) in enumerate(tiles5):
            lhs = wt_sb[glast][:, i * 256 : (i + 1) * 256].rearrange(
                "p (c f) -> p c f", c=2
            )
            nc.tensor.matmul(
                psf[:], lhs, rhs_ap(ccp, 3, kk, 3, 1),
                start=(i == 0), stop=(i == len(tiles5) - 1), perf_mode=DR,
            )
        pvf = psf[:].rearrange("p (b t) -> p b t", t=128)[:, :, :L5]
        nc.vector.tensor_reduce(
            st_sb[glast][:, 20:21], pvf, axis=mybir.AxisListType.X,
            op=mybir.AluOpType.max,
        )
        nc.vector.bn_stats(st_sb[glast][:, 21:27], pvf[:, 0, :])
        if need_min:
            nc.vector.tensor_reduce(
                st_sb[glast][:, NSTAT + 15 : NSTAT + 16], pvf,
                axis=mybir.AxisListType.X, op=mybir.AluOpType.min,
            )
        nc.sync.dma_start(stats_d[glast][:, 20:27], st_sb[glast][:, 20:27])
        if need_min:
            nc.sync.dma_start(
                stats_d[glast][:, NSTAT + 15 : nstat],
                st_sb[glast][:, NSTAT + 15 : nstat],
            )

    nc.compile()
    return nc


def _get_compiled(need_min):
    key = ("nc", need_min)
    if key not in _CACHE:
        _CACHE[key] = _build_bass(need_min)
    return _CACHE[key]


def _maybe_enable_trace():
    if os.environ.get("KERNEL_TRACE") != "1":
        return False
    try:
        import sys, types

        if "antenv.axon_hooks" not in sys.modules:
            mod = types.ModuleType("antenv.axon_hooks")
            _h = {"hook": None}
            mod.set_axon_ntff_profile_hook = lambda h: _h.__setitem__("hook", h)
            mod.get_axon_ntff_profile_hook = lambda: _h["hook"]
            sys.modules["antenv.axon_hooks"] = mod
            import antenv

            antenv.axon_hooks = mod
            from trn_agent_boot.trn_boot import _ntff_profile_via_ctypes

            mod.set_axon_ntff_profile_hook(
                _ntff_profile_via_ctypes("/opt/axon/libaxon_pjrt.so")
            )
        import concourse.bass_utils as bu

        bu.upload_artifacts = lambda tmpdir: tmpdir
        return True
    except Exception:
        return False


def _q8(a, sc):
    return np.clip(np.asarray(a, dtype=np.float32) * sc, -240.0, 240.0).astype(F8)


def kernel(
    x, emb_w,
    conv_w0, conv_b0, bn_g0, bn_b0,
    conv_w1, conv_b1, bn_g1, bn_b1,
    conv_w2, conv_b2, bn_g2, bn_b2,
    fc1_w, fc1_b, bn1_g, bn1_b, fc2_w, fc2_b,
):
    global _LAST_RESULTS
    from concourse.bass_utils import run_bass_kernel_spmd

    x = np.asarray(x, dtype=np.float32)
    emb_w = np.asarray(emb_w, dtype=np.float32)
    conv_ws = [np.asarray(w, dtype=np.float32) for w in (conv_w0, conv_w1, conv_w2)]
    bn_gs = [np.asarray(v, dtype=np.float64) for v in (bn_g0, bn_g1, bn_g2)]
    bn_bs = [np.asarray(v, dtype=np.float64) for v in (bn_b0, bn_b1, bn_b2)]
    need_min = bool((np.concatenate(bn_gs) < 0.0).any())

    # ---- host: embedding (x is one-hot in practice; dense matmul is exact) ----
    e = x.reshape(-1, V) @ emb_w                       # [B*S*W, E]
    e = e.reshape(B, S, CIN)                           # [B, S, 512]
    embT = np.ascontiguousarray(e.transpose(2, 0, 1))  # [512, B, S]
    emb8 = _q8(embT, SC_A)                             # [512, B, 128]

    # ---- pack device inputs ----
    ntiles = _weight_tile_count()
    wts = np.empty((128, ntiles * 256), dtype=F8)
    i = 0
    for bank, fc in GROUPS:
        cwq = _q8(conv_ws[bank], SC_W)                 # [256, 512, k]
        for ccp, kk in _group_tiles(bank):
            blk = cwq[fc * 128 : (fc + 1) * 128,
                      2 * ccp * 128 : (2 * ccp + 2) * 128, kk]  # [f, 2*128]
            # target [p, c*128 + f] = blk[f, c*128 + p]
            wts[:, i * 256 : (i + 1) * 256] = (
                blk.reshape(128, 2, 128).transpose(2, 1, 0).reshape(128, 256)
            )
            i += 1

    # emb8 viewed [pair, c, p, batch, t]
    ev = emb8.reshape(NPAIR, 2, 128, B, S)
    in_maps = []
    for c in range(NCORES):
        v = ev[:, :, :, c * BL : (c + 1) * BL, :].reshape(NPAIR, 2, 128, 2, 8, S)
        tmp = np.zeros((NPAIR, 128, 2, 2, XHP), dtype=F8)
        # [pair, c2, p, h, b, t] -> [pair, p, h, c2, (b t)]
        tmp[:, :, :, :, :XH] = v.transpose(0, 2, 3, 1, 4, 5).reshape(
            NPAIR, 128, 2, 2, XH
        )
        in_maps.append({"emb": tmp.reshape(NPAIR, 128, EMB_FREE), "wts": wts})

    nc = _get_compiled(need_min)
    trace = _maybe_enable_trace()
    res = run_bass_kernel_spmd(
        nc, in_maps, core_ids=list(range(NCORES)), trace=trace,
        tmpdir=os.environ.get("KERNEL_TRACE_DIR") or None,
    )
    _LAST_RESULTS = res

    # ---- host: combine stats -> BN -> pooled -> fc head (float64) ----
    FT = sum(FILTERS)  # 768
    inv = 1.0 / (SC_A * SC_W)
    cmax = np.empty((FT, B), dtype=np.float64)
    cmin = np.empty((FT, B), dtype=np.float64) if need_min else None
    sumsq = np.zeros(FT, dtype=np.float64)
    for c in range(NCORES):
        stats = res.results[c]["stats"].astype(np.float64)  # [6, 128, nstat]
        for g, (bank, fc) in enumerate(GROUPS):
            ch = bank * 256 + fc * 128
            sl = slice(ch, ch + 128)
            pieces = PIECES_LAST if g == len(GROUPS) - 1 else PIECES_FULL
            bidx = 0
            for col, nb in pieces:
                bs = slice(c * BL + bidx, c * BL + bidx + nb)
                cmax[sl, bs] = stats[g, :, col : col + nb] * inv
                if g == len(GROUPS) - 1 and col == 20:
                    bn = stats[g, :, 21:27]  # [cnt,mean,cnt*var] x even/odd
                    sq = (bn[:, 2] + bn[:, 0] * bn[:, 1] ** 2
                          + bn[:, 5] + bn[:, 3] * bn[:, 4] ** 2)
                    sumsq[sl] += sq * inv * inv
                else:
                    sumsq[sl] += stats[g, :, col + nb] * inv * inv
                if need_min:
                    cmin[sl, bs] = stats[g, :, NSTAT + bidx : NSTAT + bidx + nb] * inv
                bidx += nb

    # channel means via the factorized sum (exact: sum_t conv = w . window-sums)
    embT64 = embT.astype(np.float64)
    st_sum = embT64.sum(axis=1)                        # [512, S] summed over batch
    cum = np.concatenate(
        [np.zeros((CIN, 1)), np.cumsum(st_sum, axis=1)], axis=1
    )                                                  # [512, S+1]
    mean = np.empty(FT, dtype=np.float64)
    for bank in range(3):
        k, L = KS[bank], LS[bank]
        cw = conv_ws[bank].astype(np.float64)          # [256, 512, k]
        hs = np.stack([cum[:, kk + L] - cum[:, kk] for kk in range(k)], axis=1)
        mean[bank * 256 : (bank + 1) * 256] = (
            np.einsum("fck,ck->f", cw, hs) / (B * L)
        )

    counts = np.repeat([B * L for L in LS], FILTERS)
    var = sumsq / counts - mean * mean
    g_all = np.concatenate(bn_gs)
    b_all = np.concatenate(bn_bs)
    s = g_all / np.sqrt(var + EPS)
    shift = b_all - mean * s
    M = np.where(s[:, None] >= 0.0, cmax, cmin if need_min else cmax)  # [768, B]
    pooled = np.maximum(s[:, None] * M + shift[:, None], 0.0).T  # [B, 768]

    z = pooled @ np.asarray(fc1_w, dtype=np.float64) + np.asarray(
        fc1_b, dtype=np.float64
    )
    mu = z.mean(axis=0, keepdims=True)
    vz = np.square(z - mu).mean(axis=0, keepdims=True)
    z = (z - mu) / np.sqrt(vz + EPS) * np.asarray(
        bn1_g, dtype=np.float64
    ) + np.asarray(bn1_b, dtype=np.float64)
    z = np.maximum(z, 0.0)
    logits = z @ np.asarray(fc2_w, dtype=np.float64) + np.asarray(
        fc2_b, dtype=np.float64
    )
    logits -= logits.max(axis=1, keepdims=True)
    p = np.exp(logits)
    p /= p.sum(axis=1, keepdims=True)
    return p.astype(np.float32)
